# revision 17
# baseline (speedup 1.0000x reference)
"""Trainium2 Bass kernel for nn_ASIS_16836271801026 (gnn_message_passing).

Pipeline per NeuronCore (8 cores = 4 batches x 2 row-halves; inputs are
host-rolled so every core runs the identical program on its 2048 rows):

  1. adapted = relu(bn(adapt_w @ f_sem)); f_sins = f_ins + adapted;
     e_ins = ins_w @ f_sins                                   (PE fp32 + ACT)
  2. neg-distance Gram matrix in bf16 hi/mid/lo 3-way split (K=36, fp32-level
     accuracy) streamed through PSUM per 128-row tile       (PE bf16)
  3. top-k selection per row: segmented max8 union -> threshold tau (30th of
     union) -> value-slots replaced by their own global indices -> iterative
     max8/match_replace extracts winner indices directly     (DVE)
  4. kNN gather: SBUF-source dma_gather (striped bf16 table, 32 slots/point,
     losers remapped to a -inf pad row) -> max-tree reduce -> f_isem (GPSIMD+DVE)
  5. p_sem = sem_w @ f_isem + b                               (PE fp32)
"""
import sys
if '/opt/trn_rl_repo' not in sys.path:
    sys.path.insert(0, '/opt/trn_rl_repo')

import numpy as np
import ml_dtypes

B = 4
CH = 128          # semantic channels
N = 4096          # points
CI = 128          # instance channels
EO = 5            # ins embedding out
SO = 13           # sem out
NCORES = 8
ROWS = N // 2     # rows per core
NTILES = ROWS // 128
SEGS = 16
SEGSZ = N // SEGS
NSLOT = 32
RANKS = N // 128 + 1   # striped table ranks incl. -inf pad rank

_cache = {}


def _build_program(K):
    import concourse.bacc as bacc
    import concourse.mybir as mybir
    import concourse.tile as tile
    from concourse.alu_op_type import AluOpType
    from concourse.masks import make_identity

    DT = mybir.dt
    AF = mybir.ActivationFunctionType
    AX = mybir.AxisListType

    nc = bacc.Bacc("TRN2", target_bir_lowering=False, debug=False,
                   enable_asserts=False)

    def din(name, shape, dt=DT.float32):
        return nc.dram_tensor(name, shape, dt, kind="ExternalInput").ap()

    f_sem = din("f_sem", [CH, N])
    f_ins = din("f_ins", [CI, N])
    tabs = din("tabs", [128, RANKS * 128], DT.bfloat16)
    awT = din("awT", [CH, CI])
    bn_s = din("bn_s", [CI, 1])
    bn_tb = din("bn_tb", [CI, 1])
    iwT = din("iwT", [CI, EO])
    i_b = din("i_b", [EO, 1])
    swT = din("swT", [CH, SO])
    s_b = din("s_b", [SO, 1])
    offs1 = din("offs1", [128, SEGS * 8])
    p_out = nc.dram_tensor("p_out", [SO, ROWS], DT.float32, kind="ExternalOutput").ap()
    e_out = nc.dram_tensor("e_out", [EO, N], DT.float32, kind="ExternalOutput").ap()

    KR = 36  # gram contraction rows

    with tile.TileContext(nc, trace_sim=False) as tc:
        with (
            tc.tile_pool(name="const", bufs=1) as cpool,
            tc.tile_pool(name="big", bufs=1) as bpool,
            tc.tile_pool(name="dpool", bufs=2) as dpool,
            tc.tile_pool(name="gpool", bufs=2) as gpool,
            tc.tile_pool(name="g2pool", bufs=1) as g2pool,
            tc.tile_pool(name="upool", bufs=2) as upool,
            tc.tile_pool(name="tmp5", bufs=1) as t5pool,
            tc.tile_pool(name="psg", bufs=4, space="PSUM") as psg,
            tc.tile_pool(name="pss", bufs=1, space="PSUM") as pss,
            tc.tile_pool(name="ps5", bufs=1, space="PSUM") as ps5p,
        ):
            # ---- load constants & inputs ----
            fsem_sb = bpool.tile([CH, N], DT.float32)
            fins_sb = dpool.tile([CI, N], DT.float32, tag="d")
            tabs_sb = cpool.tile([128, RANKS * 128], DT.bfloat16)
            awT_sb = cpool.tile([CH, CI], DT.float32)
            bns_sb = cpool.tile([CI, 1], DT.float32)
            bntb_sb = cpool.tile([CI, 1], DT.float32)
            iwT_sb = cpool.tile([CI, EO], DT.float32)
            ib_sb = cpool.tile([EO, 1], DT.float32)
            swT_sb = cpool.tile([CH, SO], DT.float32)
            sb_sb = cpool.tile([SO, 1], DT.float32)
            offs_sb = cpool.tile([128, SEGS * 8], DT.float32)
            ident = cpool.tile([128, 128], DT.float32)
            ones5 = cpool.tile([EO, 1], DT.float32)

            nc.sync.dma_start(fsem_sb[:], f_sem[:])
            nc.sync.dma_start(fins_sb[:], f_ins[:])
            nc.sync.dma_start(tabs_sb[:], tabs[:])
            nc.sync.dma_start(awT_sb[:], awT[:])
            nc.sync.dma_start(bns_sb[:], bn_s[:])
            nc.sync.dma_start(bntb_sb[:], bn_tb[:])
            nc.sync.dma_start(iwT_sb[:], iwT[:])
            nc.sync.dma_start(ib_sb[:], i_b[:])
            nc.sync.dma_start(swT_sb[:], swT[:])
            nc.sync.dma_start(sb_sb[:], s_b[:])
            nc.sync.dma_start(offs_sb[:], offs1[:])
            make_identity(nc, ident[:])
            nc.vector.memset(ones5[:], 1.0)

            # ---- phase 0: embedding chain ----
            fsins = bpool.tile([CI, N], DT.float32)
            for bk in range(8):
                sl = slice(bk * 512, (bk + 1) * 512)
                ps = psg.tile([128, 512], DT.float32, tag="ps")
                nc.tensor.matmul(ps[:], awT_sb[:], fsem_sb[:, sl], start=True, stop=True)
                adch = t5pool.tile([CI, 512], DT.float32, tag="adch")
                nc.scalar.activation(adch[:], ps[:], AF.Relu,
                                     bias=bntb_sb[:, 0:1], scale=bns_sb[:, 0:1])
                nc.vector.tensor_add(fsins[:, sl], fins_sb[:, sl], adch[:])

            e_sb = dpool.tile([EO, N], DT.float32, tag="d")
            for bk in range(8):
                sl = slice(bk * 512, (bk + 1) * 512)
                p5 = ps5p.tile([EO, 512], DT.float32, tag="p5")
                nc.tensor.matmul(p5[:], iwT_sb[:], fsins[:, sl], start=True, stop=True)
                nc.scalar.activation(e_sb[:, sl], p5[:], AF.Identity,
                                     bias=ib_sb[:, 0:1], scale=1.0)
            nc.sync.dma_start(e_out[:], e_sb[:])

            # gram operand stacks (bf16 3-way split, K=36). Engine ops can only
            # address partition bases {0,32,64,96}, so split pieces are staged
            # at those bases inside shared [128,N] tiles, then DMA-assembled.
            LT = bpool.tile([KR, N], DT.bfloat16)
            RT = bpool.tile([KR, N], DT.bfloat16)
            # STG1: ehb@0, emb@32, elb@64, nsqh@96
            # STG2: eh2@0, em2@32, el2@64, nsqm@96
            # STG3: nsql@0, ones@32
            STG1 = bpool.tile([128, N], DT.bfloat16)
            STG2 = bpool.tile([128, N], DT.bfloat16)
            STG3 = bpool.tile([128, N], DT.bfloat16)
            ehb, emb, elb = STG1[0:5, :], STG1[32:37, :], STG1[64:69, :]
            nsqh = STG1[96:97, :]
            eh2, em2, el2 = STG2[0:5, :], STG2[32:37, :], STG2[64:69, :]
            nsqm = STG2[96:97, :]
            nsql = STG3[0:1, :]
            ones1 = STG3[32:33, :]
            nc.vector.memset(ones1, 1.0)

            # nsq = -sum(e^2) and its split rows, chunked to save SBUF
            for bk in range(8):
                sl = slice(bk * 512, (bk + 1) * 512)
                e2 = t5pool.tile([EO, 512], DT.float32, tag="e2")
                nc.vector.tensor_mul(e2[:], e_sb[:, sl], e_sb[:, sl])
                p1 = ps5p.tile([1, 512], DT.float32, tag="p1")
                nc.tensor.matmul(p1[:], ones5[:], e2[:], start=True, stop=True)
                nsq = t5pool.tile([1, 512], DT.float32, tag="nsq")
                nc.scalar.activation(nsq[:], p1[:], AF.Copy, bias=0.0, scale=-1.0)
                h32 = t5pool.tile([1, 512], DT.float32, tag="h32")
                s1 = t5pool.tile([1, 512], DT.float32, tag="s1")
                s2 = t5pool.tile([1, 512], DT.float32, tag="s2")
                nc.scalar.activation(nsqh[:, sl], nsq[:], AF.Copy)  # bf16 cast
                nc.scalar.activation(h32[:], nsqh[:, sl], AF.Copy)
                nc.vector.tensor_sub(s1[:], nsq[:], h32[:])
                nc.scalar.activation(nsqm[:, sl], s1[:], AF.Copy)
                nc.scalar.activation(h32[:], nsqm[:, sl], AF.Copy)
                nc.vector.tensor_sub(s2[:], s1[:], h32[:])
                nc.scalar.activation(nsql[:, sl], s2[:], AF.Copy)

            # e splits (bf16 staging, f32 temps chunked)
            for bk in range(8):
                sl = slice(bk * 512, (bk + 1) * 512)
                x32 = t5pool.tile([EO, 512], DT.float32, tag="x32")
                r1 = t5pool.tile([EO, 512], DT.float32, tag="r1")
                r2 = t5pool.tile([EO, 512], DT.float32, tag="r2")
                nc.scalar.activation(ehb[:, sl], e_sb[:, sl], AF.Copy)
                nc.scalar.activation(x32[:], ehb[:, sl], AF.Copy)
                nc.vector.tensor_sub(r1[:], e_sb[:, sl], x32[:])
                nc.vector.tensor_scalar_mul(eh2[:, sl], x32[:], 2.0)
                nc.scalar.activation(emb[:, sl], r1[:], AF.Copy)
                nc.scalar.activation(x32[:], emb[:, sl], AF.Copy)
                nc.vector.tensor_sub(r2[:], r1[:], x32[:])
                nc.vector.tensor_scalar_mul(em2[:, sl], x32[:], 2.0)
                nc.scalar.activation(elb[:, sl], r2[:], AF.Copy)
                nc.scalar.activation(x32[:], elb[:, sl], AF.Copy)
                nc.vector.tensor_scalar_mul(el2[:, sl], x32[:], 2.0)

            # assemble stacks via SBUF->SBUF DMAs (partition-arbitrary)
            # RT rows: [eh, em, eh, el, eh, em, nsqh, nsqm, nsql, 1, 1, 1]
            for dst, src in ((0, ehb), (5, emb), (10, ehb), (15, elb),
                             (20, ehb), (25, emb)):
                nc.sync.dma_start(RT[dst:dst + 5, :], src)
            for i, src in enumerate((nsqh, nsqm, nsql, ones1, ones1, ones1)):
                nc.sync.dma_start(RT[30 + i:31 + i, :], src)
            # LT rows: [2eh, 2eh, 2em, 2eh, 2el, 2em, 1, 1, 1, nsqh, nsqm, nsql]
            for dst, src in ((0, eh2), (5, eh2), (10, em2), (15, eh2),
                             (20, el2), (25, em2)):
                nc.sync.dma_start(LT[dst:dst + 5, :], src)
            for i, src in enumerate((ones1, ones1, ones1, nsqh, nsqm, nsql)):
                nc.sync.dma_start(LT[30 + i:31 + i, :], src)

            psem_strip = bpool.tile([SO, ROWS], DT.float32)
            tau_round, tau_slot = (K - 1) // 8, (K - 1) % 8

            # ---- phase 1: per row-tile ----
            for t in range(NTILES):
                tsl = slice(t * 128, (t + 1) * 128)
                D = dpool.tile([128, N], DT.float32, tag="d")
                for bk in range(8):
                    sl = slice(bk * 512, (bk + 1) * 512)
                    ps = psg.tile([128, 512], DT.float32, tag="ps")
                    nc.tensor.matmul(ps[:], LT[:, tsl], RT[:, sl], start=True, stop=True)
                    nc.scalar.activation(D[:, sl], ps[:], AF.Copy)

                M = upool.tile([128, SEGS * 8], DT.float32, tag="M")
                I = upool.tile([128, SEGS * 8], DT.uint32, tag="I")
                for s in range(SEGS):
                    nc.vector.max(M[:, 8 * s:8 * s + 8], D[:, SEGSZ * s:SEGSZ * (s + 1)])
                for s in range(SEGS):
                    nc.vector.max_index(I[:, 8 * s:8 * s + 8], M[:, 8 * s:8 * s + 8],
                                        D[:, SEGSZ * s:SEGSZ * (s + 1)])
                Mc = upool.tile([128, SEGS * 8], DT.float32, tag="Mc")
                nc.vector.tensor_copy(Mc[:], M[:])
                Vr = None
                for r in range(tau_round + 1):
                    Vr = upool.tile([128, 8], DT.float32, tag="Vr")
                    nc.vector.max(Vr[:], Mc[:])
                    if r < tau_round:
                        nc.vector.match_replace(Mc[:], Vr[:], Mc[:], imm_value=-1e30)
                tau = Vr[:, tau_slot:tau_slot + 1]

                If = upool.tile([128, SEGS * 8], DT.float32, tag="If")
                nc.vector.tensor_copy(If[:], I[:])
                Ioff = upool.tile([128, SEGS * 8], DT.float32, tag="Io")
                nc.vector.tensor_add(Ioff[:], If[:], offs_sb[:])
                Mpp = upool.tile([128, SEGS * 8], DT.float32, tag="Mp")
                nc.vector.scalar_tensor_tensor(
                    Mpp[:], M[:], tau, Ioff[:],
                    op0=AluOpType.is_ge, op1=AluOpType.mult)
                G32 = upool.tile([128, NSLOT], DT.float32, tag="G32")
                for r in range(NSLOT // 8):
                    nc.vector.max(G32[:, 8 * r:8 * r + 8], Mpp[:])
                    if r < NSLOT // 8 - 1:
                        nc.vector.match_replace(Mpp[:], G32[:, 8 * r:8 * r + 8],
                                                Mpp[:], imm_value=-2.0)
                pred = upool.tile([128, NSLOT], DT.float32, tag="pr")
                nc.vector.tensor_scalar(pred[:], G32[:], 0.5, None, op0=AluOpType.is_lt)
                gidx1 = upool.tile([128, NSLOT], DT.float32, tag="g1")
                nc.vector.scalar_tensor_tensor(
                    gidx1[:], pred[:], 4097.0, G32[:],
                    op0=AluOpType.mult, op1=AluOpType.add)
                gidx = upool.tile([128, NSLOT], DT.float32, tag="g2")
                nc.vector.tensor_scalar_sub(gidx[:], gidx1[:], 1.0)

                idx16 = upool.tile([128, NSLOT * 8], DT.int16, tag="ix")
                for h in range(NSLOT // 16):
                    grep = upool.tile([128, 128], DT.float32, tag="grep")
                    nc.vector.tensor_copy(
                        grep[:].rearrange("p (o a) -> p o a", o=8),
                        gidx[:, 16 * h:16 * (h + 1)].rearrange(
                            "p (o a) -> p o a", o=1).to_broadcast([128, 8, 16]))
                    pst = pss.tile([128, 128], DT.float32, tag="pst")
                    nc.tensor.transpose(pst[:], grep[:], ident[:])
                    nc.vector.tensor_copy(idx16[:, 128 * h:128 * (h + 1)], pst[:])

                Gt = gpool.tile([128, NSLOT * 128], DT.bfloat16, tag="gt")
                nc.gpsimd.dma_gather(
                    out_ap=Gt[:].rearrange("p (a n) -> p a n", a=1),
                    in_ap=tabs_sb[:], idxs_ap=idx16[:],
                    num_idxs=NSLOT * 128, num_idxs_reg=NSLOT * 128,
                    elem_size=128, transpose=True, single_packet=False,
                    sbuf_tokens_per_rank=128, sbuf_free_dim_per_rank=256,
                    sbuf_free_dim_pad_per_rank=0, sbuf_byte_offset=0)

                # reduce over the 32 slots: stream layout (h, point, slot16)
                H1 = gpool.tile([128, 2048], DT.bfloat16, tag="h1")
                nc.vector.tensor_max(H1[:], Gt[:, :2048], Gt[:, 2048:])
                H2 = g2pool.tile([128, 1024], DT.bfloat16, tag="h2")
                h1v = H1[:].rearrange("p (n a) -> p n a", a=16)
                nc.vector.tensor_max(H2[:].rearrange("p (n a) -> p n a", a=8),
                                     h1v[:, :, 0:8], h1v[:, :, 8:16])
                H3 = g2pool.tile([128, 512], DT.bfloat16, tag="h3")
                h2v = H2[:].rearrange("p (n a) -> p n a", a=8)
                nc.vector.tensor_max(H3[:].rearrange("p (n a) -> p n a", a=4),
                                     h2v[:, :, 0:4], h2v[:, :, 4:8])
                fiTb = upool.tile([128, 128], DT.bfloat16, tag="fb")
                nc.vector.tensor_reduce(fiTb[:], H3[:].rearrange("p (n a) -> p n a", a=4),
                                        axis=AX.X, op=AluOpType.max)
                fiT = upool.tile([128, 128], DT.float32, tag="ff")
                nc.vector.tensor_copy(fiT[:], fiTb[:])

                pp = pss.tile([SO, 128], DT.float32, tag="pp")
                nc.tensor.matmul(pp[:], swT_sb[:], fiT[:], start=True, stop=True)
                nc.scalar.activation(psem_strip[:, tsl], pp[:], AF.Identity,
                                     bias=sb_sb[:, 0:1], scale=1.0)

            nc.sync.dma_start(p_out[:], psem_strip[:])

    nc.compile()
    return nc


class _Runner:
    """Reusable SPMD PJRT runner (axon path)."""

    def __init__(self, nc, n_cores=NCORES):
        import jax
        import concourse.mybir as mybir
        from jax.sharding import Mesh, PartitionSpec
        from jax.experimental.shard_map import shard_map
        from concourse.bass2jax import (
            _bass_exec_p, install_neuronx_cc_hook, partition_id_tensor)
        install_neuronx_cc_hook()
        self.jax = jax
        self.n_cores = n_cores
        pname = nc.partition_id_tensor.name if nc.partition_id_tensor else None
        in_names, out_names, out_avals = [], [], []
        for alloc in nc.m.functions[0].allocations:
            if not isinstance(alloc, mybir.MemoryLocationSet):
                continue
            name = alloc.memorylocations[0].name
            if alloc.kind == "ExternalInput":
                if name != pname:
                    in_names.append(name)
            elif alloc.kind == "ExternalOutput":
                out_names.append(name)
                out_avals.append(jax.core.ShapedArray(
                    tuple(alloc.tensor_shape), mybir.dt.np(alloc.dtype)))
        self.in_names, self.out_names, self.out_avals = in_names, out_names, out_avals
        n_params = len(in_names)
        all_in = in_names + out_names + ([pname] if pname else [])

        def _body(*args):
            ops = list(args)
            if pname:
                ops.append(partition_id_tensor())
            return tuple(_bass_exec_p.bind(
                *ops, out_avals=tuple(out_avals), in_names=tuple(all_in),
                out_names=tuple(out_names), lowering_input_output_aliases=(),
                sim_require_finite=True, sim_require_nnan=True, nc=nc))

        devices = jax.devices()[:n_cores]
        mesh = Mesh(np.asarray(devices), ("core",))
        n_outs = len(out_names)
        self.fn = jax.jit(
            shard_map(_body, mesh=mesh,
                      in_specs=(PartitionSpec("core"),) * (n_params + n_outs),
                      out_specs=(PartitionSpec("core"),) * n_outs,
                      check_rep=False),
            keep_unused=True)

    def run(self, in_maps):
        jax = self.jax
        concat_in = [
            np.concatenate([np.asarray(in_maps[c][n]) for c in range(self.n_cores)],
                           axis=0)
            for n in self.in_names
        ]
        zeros = [np.zeros((self.n_cores * a.shape[0], *a.shape[1:]), a.dtype)
                 for a in self.out_avals]
        outs = self.fn(*concat_in, *zeros)
        jax.block_until_ready(outs)
        return [
            {n: np.asarray(outs[i]).reshape(self.n_cores, *self.out_avals[i].shape)[c]
             for i, n in enumerate(self.out_names)}
            for c in range(self.n_cores)
        ]


def _prep_core_inputs(inputs, K):
    """Host-side sharding/preprocessing. Returns list of 8 input dicts."""
    f_sem = np.asarray(inputs["f_sem"], np.float32)
    f_ins = np.asarray(inputs["f_ins"], np.float32)
    aw = np.asarray(inputs["adapt_w"], np.float32)
    ab = np.asarray(inputs["adapt_b"], np.float32)
    g = np.asarray(inputs["adapt_gamma"], np.float32)
    be = np.asarray(inputs["adapt_beta"], np.float32)
    mu = np.asarray(inputs["adapt_mean"], np.float32)
    var = np.asarray(inputs["adapt_var"], np.float32)
    iw = np.asarray(inputs["ins_w"], np.float32)
    ib = np.asarray(inputs["ins_b"], np.float32)
    sw = np.asarray(inputs["sem_w"], np.float32)
    sb = np.asarray(inputs["sem_b"], np.float32)

    inv = (1.0 / np.sqrt(var + 1e-5)).astype(np.float32)
    bn_s = (g * inv).astype(np.float32)[:, None]
    bn_tb = ((ab - mu) * g * inv + be).astype(np.float32)[:, None]
    awT = np.ascontiguousarray(aw.T)
    iwT = np.ascontiguousarray(iw.T)
    swT = np.ascontiguousarray(sw.T)
    ib2 = ib.astype(np.float32)[:, None]
    sb2 = sb.astype(np.float32)[:, None]
    q = np.arange(SEGS * 8)
    offs1 = np.broadcast_to(((q // 8) * SEGSZ + 1).astype(np.float32)[None, :],
                            (128, SEGS * 8)).copy()

    maps = []
    for d in range(NCORES):
        b, half = d // 2, d % 2
        r0 = half * ROWS
        fs = np.roll(f_sem[b], -r0, axis=1) if r0 else f_sem[b]
        fi = np.roll(f_ins[b], -r0, axis=1) if r0 else f_ins[b]
        fs = np.ascontiguousarray(fs)
        fi = np.ascontiguousarray(fi)
        # striped bf16 table: token j -> partition j%128, rank j//128
        tabT = fs.T.astype(ml_dtypes.bfloat16)        # [N, 128]
        tabs = np.full((128, RANKS, 128), -3.0e38, ml_dtypes.bfloat16)
        tabs[:, :N // 128, :] = tabT.reshape(N // 128, 128, 128).transpose(1, 0, 2)
        maps.append({
            "f_sem": fs, "f_ins": fi,
            "tabs": np.ascontiguousarray(tabs.reshape(128, RANKS * 128)),
            "awT": awT, "bn_s": bn_s, "bn_tb": bn_tb,
            "iwT": iwT, "i_b": ib2, "swT": swT, "s_b": sb2,
            "offs1": offs1,
        })
    return maps


def kernel(**inputs):
    K = int(np.asarray(inputs["k"]))
    assert 9 <= K <= NSLOT, f"kernel compiled for k in [9,{NSLOT}], got {K}"
    if K not in _cache:
        nc = _build_program(K)
        _cache[K] = (nc, _Runner(nc))
    nc, runner = _cache[K]
    maps = _prep_core_inputs(inputs, K)
    res = runner.run(maps)

    p_sem = np.zeros((B, SO, N), np.float32)
    e_ins = np.zeros((B, EO, N), np.float32)
    for d in range(NCORES):
        b, half = d // 2, d % 2
        r0 = half * ROWS
        p_sem[b, :, r0:r0 + ROWS] = res[d]["p_out"]
        if half == 0:
            e_ins[b] = res[d]["e_out"]
    return p_sem, e_ins
